# revision 39
# baseline (speedup 1.0000x reference)
"""Direct N-body gravitational acceleration on 8 Trainium2 NeuronCores.

Sharding: target-particle axis j split across the 8 cores (1024 targets
per core); every core holds the full (N,3) source positions.

Math (reference):
    z[i,j]   = |y_i - y_j|^2 + eps
    icd[i,j] = z^{-3/2}
    acc[j]   = G*m_j * (sum_i icd[i,j]*y_i  -  y_j * sum_i icd[i,j])

Per-core device pipeline (everything O(N^2) stays on-chip):
    mm1 (PE): z[i,j] = a_i . b_j with a_i=(y_i, d2_i, 1),
        b_j=(-2*y_j, 1, d2_j+eps).  To avoid the 4x-cost fp32 matmul
        path each fp32 feature is 3-way fp16 split and the product
        expanded into 6 cross terms -> one K=30 fp16 matmul with
        fp32-grade accuracy.  i-tiles alternate between PE row strips
        0-29 / 32-61 (tile_position row tiling) so consecutive i-tiles'
        matmuls run concurrently.
    ACT: t = Abs_reciprocal_sqrt(s*z) with s = ICD_SCALE^(2/3), so that
        t^3 = icd/ICD_SCALE directly (no separate clamp/scale pass).
        z lives in a 6-slot x [128,512] PSUM rotation (3 banks per
        ping-pong half): each ACTIVATE covers 3 slots (FD=1536,
        amortizing the ~0.5us per-call ACT overhead) while mm1 fills
        the other 3 slots -> full overlap.
    DVE: icd = t^3 (fp16 in / fp16 out) at 4 elements/cycle/lane: the
        per-NEFF DVE table repoints the stock TENSOR_SCALAR_ARITH
        opcode row at hand-written cube uop programs (one per perf
        mode), and the kernel issues plain tensor_scalar_mul
        instructions; the engine's RTL mode-detect sees fp16/step-1/
        SBUF/single-src and runs the 4x_2PORT program.
    mm2 (PE, fp16, K=128 per i-tile, PSUM-accumulated over 64 i-tiles):
        S[c,j] += sum_i yp[i,c]*icd[i,j], with yp = [y | 1] split into
        fp16 hi+lo halves packed as one [128, 8] weight (the y_i - y_j
        cancellation must survive quantization); the two j-halves run
        concurrently in PE column strips 0 / 32.
Host does the O(N) prep (feature splits) and the O(N) affine combine
    acc[j] = ICD_SCALE*G*m_j*(S[0:3,j] - y_j*S[3,j]) with S = S_hi+S_lo.
"""

import numpy as np

N = 8192
NCORES = 8
JL = N // NCORES  # 1024 local targets per core
P = 128
ITILES = N // P  # 64
NH = 2 * ITILES  # 128 half-tiles of [128, 512] z values
EPS = np.float32(0.01 * 0.01)
ICD_SCALE = 32.0  # icd stored as icd/32 in fp16 (max ~39k < 65504)
ACT_SCALE = float(ICD_SCALE ** (2.0 / 3.0))  # rsqrt(s*z)^3 = icd/ICD_SCALE
KF = 30  # feature rows after 3-way fp16 split (6 kept cross-product pairs)
NSLOT = 6  # z rotation slots of [128, 512] fp32 (1 PSUM bank each)
HPA = 3  # half-tiles per ACTIVATE call (FD = 1536)

CUBE_OP_NAME = "CUBE43_NB"
TS_ARITH_OPCODE = 0x43  # NEURON_ISA_TPB_OPCODE_TENSOR_SCALAR_ARITH_OP

_cache: dict = {}
LAST_RUN = None  # BassKernelResults of the most recent launch (for test.py)


def _build_cube_uops():
    """Hand-authored uop programs for out = in0^3 (fp16), one per DVE
    perf mode, mirroring the stock tensor_scalar (0x43) slot structure.

    Each element needs 2 ALU stages (square, then multiply by the
    carried input), so 1/2/4 copies fit the 8-slice pipe.  Crossbar
    lanes per the mode contract: SRC_0/SRC_0_HI are the packed fp16
    pair on read port 0, SRC_1/SRC_1_HI the pair on port 1 (single-
    tensor double-pump in the 2-port modes).  Results ride delay chains
    (or the ALU bypass chain) to the output muxes, exactly as the stock
    mode programs do."""
    from concourse.dve_uop import (
        AluInp,
        AluOp,
        DelayInp,
        InpSel,
        OutPath,
        OutSel,
        Trigger,
        UopConfig,
    )

    MUL = AluOp.MULTIPLY
    PREV = AluInp.PREV_ALU_OUT
    D = [
        AluInp.PREV_DELAY_0,
        AluInp.PREV_DELAY_1,
        AluInp.PREV_DELAY_2,
        AluInp.PREV_DELAY_3,
        AluInp.PREV_DELAY_4,
        AluInp.PREV_DELAY_5,
    ]
    SRC_DONE = (Trigger.SRC_TENSOR_DONE, Trigger.NONE, Trigger.NONE)

    def base(reads_src1):
        u = UopConfig()
        u.require_inp0 = 1
        u.require_inp1 = 1 if reads_src1 else 0
        u.trigger = SRC_DONE
        return u

    # 1x: x on input lane 0 (block-0 ALU direct) + chain 0 (dup for reuse)
    u1 = base(False)
    u1.enable_input(InpSel.SRC_0, 0)
    u1.enable_input(InpSel.SRC_0, 1)
    dp = u1.datapath_config
    dp[0].enable_alu(MUL, PREV, PREV).pass_through_delay(0)
    dp[1].enable_alu(MUL, PREV, D[0])
    for k in range(2, 8):
        dp[k].pass_through_alu()
    u1.enable_output(OutSel.ALU_OUT, OutPath.WR0_LO)

    def two_x(hi_sel, out_paths):
        # element 0 at lane0+chain0, element 1 at chain2 (stock slot pos)
        u = base(hi_sel == InpSel.SRC_1)
        u.enable_input(InpSel.SRC_0, 0)
        u.enable_input(InpSel.SRC_0, 1)
        u.enable_input(hi_sel, 3)
        dp = u.datapath_config
        dp[0].enable_alu(MUL, PREV, PREV).pass_through_delay(0, 2)  # x0^2
        dp[1].enable_alu(MUL, PREV, D[0]).pass_through_delay(2)  # res0
        dp[2].enable_alu(MUL, D[2], D[2]).pass_through_delay(2)  # x1^2
        dp[2].enable_delay_from_src(DelayInp.PREV_ALU_OUT, 1)  # cap res0
        dp[3].enable_alu(MUL, PREV, D[2]).pass_through_delay(1)  # res1
        for k in range(4, 8):
            dp[k].pass_through_alu().pass_through_delay(1)
        u.enable_output(OutSel.DELAY_1, out_paths[0])  # res0
        u.enable_output(OutSel.ALU_OUT, out_paths[1])  # res1
        return u

    u2 = two_x(InpSel.SRC_0_HI, (OutPath.WR0_LO, OutPath.WR0_HI))
    u2p = two_x(InpSel.SRC_1, (OutPath.WR0_LO, OutPath.WR1_LO))

    # 4x: e0=lane0+chain0, e1=SRC_0_HI chain2, e2=SRC_1 chain3,
    # e3=SRC_1_HI chain4; results parked in chains 1, 0, 2 + ALU_OUT
    u4 = base(True)
    u4.enable_input(InpSel.SRC_0, 0)
    u4.enable_input(InpSel.SRC_0, 1)
    u4.enable_input(InpSel.SRC_0_HI, 3)
    u4.enable_input(InpSel.SRC_1, 4)
    u4.enable_input(InpSel.SRC_1_HI, 5)
    dp = u4.datapath_config
    dp[0].enable_alu(MUL, PREV, PREV).pass_through_delay(0, 2, 3, 4)
    dp[1].enable_alu(MUL, PREV, D[0]).pass_through_delay(2, 3, 4)  # res0
    dp[2].enable_alu(MUL, D[2], D[2]).pass_through_delay(2, 3, 4)
    dp[2].enable_delay_from_src(DelayInp.PREV_ALU_OUT, 1)  # res0 -> c1
    dp[3].enable_alu(MUL, PREV, D[2]).pass_through_delay(1, 3, 4)  # res1
    dp[4].enable_alu(MUL, D[3], D[3]).pass_through_delay(1, 3, 4)
    dp[4].enable_delay_from_src(DelayInp.PREV_ALU_OUT, 0)  # res1 -> c0
    dp[5].enable_alu(MUL, PREV, D[3]).pass_through_delay(0, 1, 4)  # res2
    dp[6].enable_alu(MUL, D[4], D[4]).pass_through_delay(0, 1, 4)
    dp[6].enable_delay_from_src(DelayInp.PREV_ALU_OUT, 2)  # res2 -> c2
    dp[7].enable_alu(MUL, PREV, D[4]).pass_through_delay(0, 1, 2)  # res3
    u4.enable_output(OutSel.DELAY_1, OutPath.WR0_LO)  # res0
    u4.enable_output(OutSel.DELAY_0, OutPath.WR0_HI)  # res1
    u4.enable_output(OutSel.DELAY_2, OutPath.WR1_LO)  # res2
    u4.enable_output(OutSel.ALU_OUT, OutPath.WR1_HI)  # res3

    return u1, u2, u2p, u4


def _register_cube_op():
    """Register out = in0^3 over the stock TENSOR_SCALAR_ARITH opcode row.

    The per-NEFF DVE table generator (the documented dve_ops extension
    point, applied at runtime since the repo is read-only) repoints
    opcode_table[0x43] at our four mode programs; the stock entry's old
    slots are left orphaned, which the generator explicitly supports.
    The kernel then issues plain `tensor_scalar_mul(out, t, 1.0)`
    instructions: the engine's RTL mode-detect sees fp16/step-1/SBUF/
    single-src and picks the 4x program — 4 elements/cycle/lane."""
    import concourse.dve_ops as dve_ops
    from concourse.dve_spec import Spec, Src0, sq
    from concourse.dve_uop import DveOpSpec

    for op in dve_ops.OPS:
        if op.name == CUBE_OP_NAME:
            return op

    spec = Spec(
        body=sq(Src0) * Src0,
        reference=lambda in0, in1, s0, s1, imm2: in0.astype(np.float32) ** 3,
    )
    u1, u2, u2p, u4 = _build_cube_uops()
    compiled = DveOpSpec(
        name=CUBE_OP_NAME,
        opcode=TS_ARITH_OPCODE,
        uops=[u1],
        uops_2x=[u2],
        uops_2x_2p=[u2p],
        uops_4x=[u4],
        perf_max=3,
        rd1_en=False,
    )
    compiled.validate("v3")
    op = dve_ops.DveOp(
        CUBE_OP_NAME, spec, subdim=False, uops_sha={"v3": compiled.sha("v3")}
    )
    dve_ops.OPS.append(op)
    dve_ops.CUSTOM_DVE_SPECS[CUBE_OP_NAME] = spec
    dve_ops._SUB_OPCODE_FOR_NAME[CUBE_OP_NAME] = 0x1F  # unused ant row
    # Pre-seed the compile cache so DveOp.compile() (used by the NEFF DVE
    # table writer) returns the spec with the mode variants + 0x43 row.
    dve_ops._COMPILE_CACHE[(CUBE_OP_NAME, "v3")] = compiled
    return op


def _build():
    import concourse.bacc as bacc
    import concourse.mybir as mybir
    import concourse.tile as tile

    f32 = mybir.dt.float32
    f16 = mybir.dt.float16
    AF = mybir.ActivationFunctionType

    _register_cube_op()

    nc = bacc.Bacc("TRN2", target_bir_lowering=False, debug=False)
    # aTP packs the K=30 feature panels of groups of 4 i-tiles into row
    # strips at partitions 0-29/32-61/64-93/96-125 so 4 consecutive
    # i-tiles' mm1 matmuls run concurrently in different PE row groups
    # (tile_position row tiling) — the K=30 contraction only needs a
    # quarter of the 128-row array.
    aTP = nc.dram_tensor("aTP", [P, (N // 2)], f16, kind="ExternalInput")
    bTP = nc.dram_tensor("bTP", [P, JL], f16, kind="ExternalInput")
    ypc = nc.dram_tensor("ypc", [N, 8], f16, kind="ExternalInput")
    S = nc.dram_tensor("S", [8, JL], f32, kind="ExternalOutput")

    NACT = (NH + HPA - 1) // HPA  # 43 ACTIVATE/cube calls (42 full + 1 tail)

    with tile.TileContext(nc) as tc:
        with (
            tc.tile_pool(name="const", bufs=1) as cpool,
            tc.tile_pool(name="tp", bufs=4) as tpool,
            tc.tile_pool(name="icdp", bufs=4) as icdpool,
            tc.tile_pool(name="zp", bufs=1, space="PSUM") as zpool,
            tc.tile_pool(name="ps2", bufs=1, space="PSUM") as ps2pool,
            tc.tile_pool(name="scr", bufs=1, space="PSUM") as scrpool,
            tc.tile_pool(name="outp", bufs=1) as opool,
        ):
            # PE warm-up: the memset is the only dependency, so the burst
            # starts as soon as the vector queue comes up and trips the HAM
            # clock gate to 8/8 (2.4 GHz) while the input DMAs land.
            warm_in = cpool.tile([P, 512], f16)
            nc.gpsimd.memset(warm_in[:], 0.0)
            scr = scrpool.tile([P, 512], f32)
            for w in range(8):
                nc.tensor.matmul(
                    scr[:], warm_in[:, 0:128], warm_in[:], start=True, stop=True
                )

            # DMA order tracks first use: the earliest mm1 halves need
            # aTP's first column chunk and bTP's first j-half only.
            aTP_sb = cpool.tile([P, N // 2], f16)
            bTP_sb = cpool.tile([P, JL], f16)
            nc.sync.dma_start(aTP_sb[:, 0 : N // 8], aTP[:, 0 : N // 8])
            nc.sync.dma_start(bTP_sb[:, 0:512], bTP[:, 0:512])
            nc.sync.dma_start(bTP_sb[:, 512:1024], bTP[:, 512:1024])
            # ypc before the late aTP chunks: the first mm2's LDWEIGHTS
            # gates on it at ~15us, and a late ypc once stalled the PE
            # queue >3.4us — tripping the HAM re-throttle death spiral.
            ypc_sb = cpool.tile([P, ITILES, 8], f16)
            nc.sync.dma_start(ypc_sb[:], ypc.rearrange("(t p) c -> p t c", p=P))
            for q in range(1, 4):
                lo, hi = q * (N // 8), (q + 1) * (N // 8)
                nc.sync.dma_start(aTP_sb[:, lo:hi], aTP[:, lo:hi])

            # 6-slot z rotation: two ping-pong tiles of 3 PSUM banks each.
            # Separate tiles (not one [128,3072] buffer) because the WAR
            # write-after-read hazard vs the ACTIVATE reads is tracked at
            # tile granularity: with one tile every mm1 write would wait
            # for the latest ACT call and serialize the whole pipeline.
            zbufA = zpool.tile([P, HPA * 512], f32)
            zbufB = zpool.tile([P, HPA * 512], f32)
            zbufs = [zbufA, zbufB]
            # both j-half accumulators share one PSUM bank: rows 0-7 hold
            # the first half, rows 32-39 the second (col tile_position 32)
            ps2 = ps2pool.tile([P, 512], f32)
            ps2a = ps2[0:8, :]
            ps2b = ps2[32:40, :]

            def emit_pulse_mm():
                # short full-array pulse: keeps HAM activity up through the
                # bubbly fill phase without inflating the cold-clock period
                nc.tensor.matmul(
                    scr[:, 0:128], warm_in[:, 0:128], warm_in[:, 0:128],
                    start=True, stop=True,
                )

            def emit_dummy_mm():
                # Dependency-free filler matmul.  The HAM clock gate
                # re-throttles the PE to 1.2 GHz whenever its busy fraction
                # drops; the real mm1+mm2 work alone leaves the warm-clock PE
                # ~60% busy, which re-throttles it and doubles the PE's share
                # until it becomes the pipeline bottleneck.  Padding the PE
                # queue keeps the duty cycle high at 2.4 GHz — a large net
                # win (measured on the previous revision of this kernel).
                nc.tensor.matmul(
                    scr[:], warm_in[:, 0:128], warm_in[:], start=True, stop=True
                )

            def emit_mm1_half(h):
                # z for half-tile h (i-tile h//2, j-half h%2): one K=30
                # matmul into its z slot.  Row strip cycles with HALF mod 4
                # (each i-tile's feature panel is replicated in two strips)
                # so any 4 consecutive halves overlap in the PE array —
                # including the two j-halves of one i-tile.
                jh = h % 2
                r = h % 4
                lhs = aTP_sb[32 * r : 32 * r + KF, (h // 4) * P : (h // 4 + 1) * P]
                zb = zbufs[(h % NSLOT) // HPA]
                s = h % HPA
                nc.tensor.matmul(
                    zb[:, s * 512 : (s + 1) * 512],
                    lhs,
                    bTP_sb[32 * r : 32 * r + KF, jh * 512 : (jh + 1) * 512],
                    start=True,
                    stop=True,
                    tile_position=(32 * r, 0),
                )

            # icd half-tile h lives at pieces[h] = (tile, column offset)
            pieces: dict[int, tuple] = {}
            next_mm1 = 0  # next half-tile whose mm1 has not been emitted
            next_mm2 = 0  # next i-tile whose mm2 has not been emitted

            # fill: z for the first ACT call plus the other ping-pong half
            while next_mm1 < NSLOT:
                emit_mm1_half(next_mm1)
                next_mm1 += 1

            for k in range(NACT):
                h0 = k * HPA
                nh = min(HPA, NH - h0)  # 3, except 2 on the tail call
                fd = nh * 512
                t_sb = tpool.tile([P, HPA * 512], f16, tag="t")
                nc.scalar.activation(
                    t_sb[:, 0:fd],
                    zbufs[k % 2][:, 0:fd],
                    AF.Abs_reciprocal_sqrt,
                    scale=ACT_SCALE,
                )
                icd = icdpool.tile([P, HPA * 512], f16, tag="icd")
                # lowers to opcode 0x43, whose table row now holds the cube
                nc.vector.tensor_scalar_mul(icd[:, 0:fd], t_sb[:, 0:fd], 1.0)
                for i in range(nh):
                    pieces[h0 + i] = (icd, i * 512)
                # fill only the ping-pong half freed by ACT call k-1: a
                # deeper lookahead would queue matmuls that WAR-depend on
                # the ACT call just issued, and the in-order PE queue then
                # head-of-line-blocks every instruction behind them.  They
                # go ahead of the dummy/mm2 emissions: the next ACT call
                # gates on them.
                while next_mm1 < NH and next_mm1 <= h0 + nh - 1 + HPA:
                    emit_mm1_half(next_mm1)
                    next_mm1 += 1
                # mm2 for every i-tile whose both halves now have icd
                while next_mm2 < ITILES and 2 * next_mm2 + 1 <= h0 + nh - 1:
                    t = next_mm2
                    ia, oa = pieces.pop(2 * t)
                    ib, ob = pieces.pop(2 * t + 1)
                    first, last = t == 0, t == ITILES - 1
                    nc.tensor.matmul(
                        ps2a,
                        ypc_sb[:, t, :],
                        ia[:, oa : oa + 512],
                        start=first,
                        stop=last,
                        tile_position=(0, 0),
                    )
                    nc.tensor.matmul(
                        ps2b,
                        ypc_sb[:, t, :],
                        ib[:, ob : ob + 512],
                        start=first,
                        stop=last,
                        tile_position=(0, 32),
                    )
                    next_mm2 += 1
                emit_dummy_mm()
                if k < 8:
                    # denser padding through the pipeline-fill phase: the
                    # HAM gate is most likely to re-throttle in the bubbly
                    # transition right after the warm-up burst
                    emit_pulse_mm()
                    emit_pulse_mm()

            S_sb = opool.tile([8, JL], f32)
            nc.vector.tensor_copy(S_sb[:, 0:512], ps2a)
            nc.sync.dma_start(S[:, 0:512], S_sb[:, 0:512])
            nc.vector.tensor_copy(S_sb[:, 512:1024], ps2b)
            nc.sync.dma_start(S[:, 512:1024], S_sb[:, 512:1024])
    # the cube rides the stock tensor_scalar opcode, so no custom-DVE
    # instruction records the op name — attach it explicitly so the NEFF
    # DVE table writer emits our 0x43 row.
    nc.m.ant_custom_dve_ops = sorted({*nc.m.ant_custom_dve_ops, CUBE_OP_NAME})
    nc.compile()
    return nc


def _split16(x):
    hi = x.astype(np.float16)
    lo = (x - hi.astype(np.float32)).astype(np.float16)
    return hi, lo


def _split16_3(x):
    h = x.astype(np.float16)
    r = x - h.astype(np.float32)
    m = r.astype(np.float16)
    l = (r - m.astype(np.float32)).astype(np.float16)
    return h, m, l


def kernel(t, y, masses, G):
    global LAST_RUN
    from concourse.bass_utils import run_bass_kernel_spmd

    y = np.asarray(y, np.float32).reshape(N, 3)
    m = np.asarray(masses, np.float32).reshape(N)
    g = np.float32(np.asarray(G).reshape(()))

    d2 = (y * y).sum(1, dtype=np.float32)
    ones = np.ones(N, np.float32)
    a = np.stack([y[:, 0], y[:, 1], y[:, 2], d2, ones])  # [5, N] fp32
    b = np.stack([-2 * y[:, 0], -2 * y[:, 1], -2 * y[:, 2], ones, d2 + EPS])
    ah, am, al = _split16_3(a)
    bh, bm, bl = _split16_3(b)
    # (ah+am+al).(bh+bm+bl) expanded, keeping pairs whose product can reach
    # ~2^-22 of z: (h,h) (h,m) (m,h) (h,l) (l,h) (m,m); dropped terms < 2^-33.
    aT30 = np.concatenate([ah, ah, am, ah, al, am], axis=0)  # [30, N]
    bT30_full = np.concatenate([bh, bm, bh, bl, bh, bm], axis=0)  # [30, N]
    # pack the feature panel for half-tile h into row strip h%4, column
    # block h//4 (partitions 0-29 / 32-61 / 64-93 / 96-125); each i-tile's
    # panel lands in two strips, one per j-half, so any 4 consecutive
    # halves run concurrently in the PE array
    NHALF = 2 * ITILES
    aTP = np.zeros((P, (NHALF // 4) * P), np.float16)
    aTP_v = aTP.reshape(P, NHALF // 4, P)
    for h in range(NHALF):
        r, blk = h % 4, h // 4
        t = h // 2
        aTP_v[32 * r : 32 * r + KF, blk] = aT30[:, t * P : (t + 1) * P]
    yp = np.concatenate([y, ones[:, None]], axis=1)  # [N, 4] fp32
    yph, ypl = _split16(yp)
    ypc = np.ascontiguousarray(np.concatenate([yph, ypl], axis=1))  # [N, 8]

    if "nc" not in _cache:
        _cache["nc"] = _build()
    nc = _cache["nc"]

    in_maps = []
    for c in range(NCORES):
        bT_loc = bT30_full[:, c * JL : (c + 1) * JL]
        bTP = np.zeros((P, JL), np.float16)
        for r in range(4):
            bTP[32 * r : 32 * r + KF] = bT_loc
        in_maps.append({"aTP": aTP, "bTP": bTP, "ypc": ypc})
    LAST_RUN = run_bass_kernel_spmd(nc, in_maps, core_ids=list(range(NCORES)))
    S8 = np.concatenate([r["S"] for r in LAST_RUN.results], axis=1)  # [8, N]
    S = S8[0:4] + S8[4:8]
    acc = (np.float32(ICD_SCALE) * g * m)[:, None] * (
        S[0:3].T - y * S[3][:, None]
    )
    return acc.astype(np.float32)


# revision 40
# speedup vs baseline: 1.0400x; 1.0400x over previous
"""Direct N-body gravitational acceleration on 8 Trainium2 NeuronCores.

Sharding: target-particle axis j split across the 8 cores (1024 targets
per core); every core holds the full (N,3) source positions.

Math (reference):
    z[i,j]   = |y_i - y_j|^2 + eps
    icd[i,j] = z^{-3/2}
    acc[j]   = G*m_j * (sum_i icd[i,j]*y_i  -  y_j * sum_i icd[i,j])

Per-core device pipeline (everything O(N^2) stays on-chip):
    mm1 (PE): z[i,j] = a_i . b_j with a_i=(y_i, d2_i, 1),
        b_j=(-2*y_j, 1, d2_j+eps).  To avoid the 4x-cost fp32 matmul
        path each fp32 feature is 3-way fp16 split and the product
        expanded into 6 cross terms -> one K=30 fp16 matmul with
        fp32-grade accuracy.  i-tiles alternate between PE row strips
        0-29 / 32-61 (tile_position row tiling) so consecutive i-tiles'
        matmuls run concurrently.
    ACT: t = Abs_reciprocal_sqrt(s*z) with s = ICD_SCALE^(2/3), so that
        t^3 = icd/ICD_SCALE directly (no separate clamp/scale pass).
        z lives in a 6-slot x [128,512] PSUM rotation (3 banks per
        ping-pong half): each ACTIVATE covers 3 slots (FD=1536,
        amortizing the ~0.5us per-call ACT overhead) while mm1 fills
        the other 3 slots -> full overlap.
    DVE: icd = t^3 (fp16 in / fp16 out) at 4 elements/cycle/lane: the
        per-NEFF DVE table repoints the stock TENSOR_SCALAR_ARITH
        opcode row at hand-written cube uop programs (one per perf
        mode), and the kernel issues plain tensor_scalar_mul
        instructions; the engine's RTL mode-detect sees fp16/step-1/
        SBUF/single-src and runs the 4x_2PORT program.
    mm2 (PE, fp16, K=128 per i-tile, PSUM-accumulated over 64 i-tiles):
        S[c,j] += sum_i yp[i,c]*icd[i,j], with yp = [y | 1] split into
        fp16 hi+lo halves packed as one [128, 8] weight (the y_i - y_j
        cancellation must survive quantization); the two j-halves run
        concurrently in PE column strips 0 / 32.
Host does the O(N) prep (feature splits) and the O(N) affine combine
    acc[j] = ICD_SCALE*G*m_j*(S[0:3,j] - y_j*S[3,j]) with S = S_hi+S_lo.
"""

import numpy as np

N = 8192
NCORES = 8
JL = N // NCORES  # 1024 local targets per core
P = 128
ITILES = N // P  # 64
NH = 2 * ITILES  # 128 half-tiles of [128, 512] z values
EPS = np.float32(0.01 * 0.01)
ICD_SCALE = 32.0  # icd stored as icd/32 in fp16 (max ~39k < 65504)
ACT_SCALE = float(ICD_SCALE ** (2.0 / 3.0))  # rsqrt(s*z)^3 = icd/ICD_SCALE
KF = 30  # feature rows after 3-way fp16 split (6 kept cross-product pairs)
NSLOT = 6  # z rotation slots of [128, 512] fp32 (1 PSUM bank each)
HPA = 3  # half-tiles per ACTIVATE call (FD = 1536)

CUBE_OP_NAME = "CUBE43_NB"
TS_ARITH_OPCODE = 0x43  # NEURON_ISA_TPB_OPCODE_TENSOR_SCALAR_ARITH_OP

_cache: dict = {}
LAST_RUN = None  # BassKernelResults of the most recent launch (for test.py)


def _build_cube_uops():
    """Hand-authored uop programs for out = in0^3 (fp16), one per DVE
    perf mode, mirroring the stock tensor_scalar (0x43) slot structure.

    Each element needs 2 ALU stages (square, then multiply by the
    carried input), so 1/2/4 copies fit the 8-slice pipe.  Crossbar
    lanes per the mode contract: SRC_0/SRC_0_HI are the packed fp16
    pair on read port 0, SRC_1/SRC_1_HI the pair on port 1 (single-
    tensor double-pump in the 2-port modes).  Results ride delay chains
    (or the ALU bypass chain) to the output muxes, exactly as the stock
    mode programs do."""
    from concourse.dve_uop import (
        AluInp,
        AluOp,
        DelayInp,
        InpSel,
        OutPath,
        OutSel,
        Trigger,
        UopConfig,
    )

    MUL = AluOp.MULTIPLY
    PREV = AluInp.PREV_ALU_OUT
    D = [
        AluInp.PREV_DELAY_0,
        AluInp.PREV_DELAY_1,
        AluInp.PREV_DELAY_2,
        AluInp.PREV_DELAY_3,
        AluInp.PREV_DELAY_4,
        AluInp.PREV_DELAY_5,
    ]
    SRC_DONE = (Trigger.SRC_TENSOR_DONE, Trigger.NONE, Trigger.NONE)

    def base(reads_src1):
        u = UopConfig()
        u.require_inp0 = 1
        u.require_inp1 = 1 if reads_src1 else 0
        u.trigger = SRC_DONE
        return u

    # 1x: x on input lane 0 (block-0 ALU direct) + chain 0 (dup for reuse)
    u1 = base(False)
    u1.enable_input(InpSel.SRC_0, 0)
    u1.enable_input(InpSel.SRC_0, 1)
    dp = u1.datapath_config
    dp[0].enable_alu(MUL, PREV, PREV).pass_through_delay(0)
    dp[1].enable_alu(MUL, PREV, D[0])
    for k in range(2, 8):
        dp[k].pass_through_alu()
    u1.enable_output(OutSel.ALU_OUT, OutPath.WR0_LO)

    def two_x(hi_sel, out_paths):
        # element 0 at lane0+chain0, element 1 at chain2 (stock slot pos)
        u = base(hi_sel == InpSel.SRC_1)
        u.enable_input(InpSel.SRC_0, 0)
        u.enable_input(InpSel.SRC_0, 1)
        u.enable_input(hi_sel, 3)
        dp = u.datapath_config
        dp[0].enable_alu(MUL, PREV, PREV).pass_through_delay(0, 2)  # x0^2
        dp[1].enable_alu(MUL, PREV, D[0]).pass_through_delay(2)  # res0
        dp[2].enable_alu(MUL, D[2], D[2]).pass_through_delay(2)  # x1^2
        dp[2].enable_delay_from_src(DelayInp.PREV_ALU_OUT, 1)  # cap res0
        dp[3].enable_alu(MUL, PREV, D[2]).pass_through_delay(1)  # res1
        for k in range(4, 8):
            dp[k].pass_through_alu().pass_through_delay(1)
        u.enable_output(OutSel.DELAY_1, out_paths[0])  # res0
        u.enable_output(OutSel.ALU_OUT, out_paths[1])  # res1
        return u

    u2 = two_x(InpSel.SRC_0_HI, (OutPath.WR0_LO, OutPath.WR0_HI))
    u2p = two_x(InpSel.SRC_1, (OutPath.WR0_LO, OutPath.WR1_LO))

    # 4x: e0=lane0+chain0, e1=SRC_0_HI chain2, e2=SRC_1 chain3,
    # e3=SRC_1_HI chain4; results parked in chains 1, 0, 2 + ALU_OUT
    u4 = base(True)
    u4.enable_input(InpSel.SRC_0, 0)
    u4.enable_input(InpSel.SRC_0, 1)
    u4.enable_input(InpSel.SRC_0_HI, 3)
    u4.enable_input(InpSel.SRC_1, 4)
    u4.enable_input(InpSel.SRC_1_HI, 5)
    dp = u4.datapath_config
    dp[0].enable_alu(MUL, PREV, PREV).pass_through_delay(0, 2, 3, 4)
    dp[1].enable_alu(MUL, PREV, D[0]).pass_through_delay(2, 3, 4)  # res0
    dp[2].enable_alu(MUL, D[2], D[2]).pass_through_delay(2, 3, 4)
    dp[2].enable_delay_from_src(DelayInp.PREV_ALU_OUT, 1)  # res0 -> c1
    dp[3].enable_alu(MUL, PREV, D[2]).pass_through_delay(1, 3, 4)  # res1
    dp[4].enable_alu(MUL, D[3], D[3]).pass_through_delay(1, 3, 4)
    dp[4].enable_delay_from_src(DelayInp.PREV_ALU_OUT, 0)  # res1 -> c0
    dp[5].enable_alu(MUL, PREV, D[3]).pass_through_delay(0, 1, 4)  # res2
    dp[6].enable_alu(MUL, D[4], D[4]).pass_through_delay(0, 1, 4)
    dp[6].enable_delay_from_src(DelayInp.PREV_ALU_OUT, 2)  # res2 -> c2
    dp[7].enable_alu(MUL, PREV, D[4]).pass_through_delay(0, 1, 2)  # res3
    u4.enable_output(OutSel.DELAY_1, OutPath.WR0_LO)  # res0
    u4.enable_output(OutSel.DELAY_0, OutPath.WR0_HI)  # res1
    u4.enable_output(OutSel.DELAY_2, OutPath.WR1_LO)  # res2
    u4.enable_output(OutSel.ALU_OUT, OutPath.WR1_HI)  # res3

    return u1, u2, u2p, u4


def _register_cube_op():
    """Register out = in0^3 over the stock TENSOR_SCALAR_ARITH opcode row.

    The per-NEFF DVE table generator (the documented dve_ops extension
    point, applied at runtime since the repo is read-only) repoints
    opcode_table[0x43] at our four mode programs; the stock entry's old
    slots are left orphaned, which the generator explicitly supports.
    The kernel then issues plain `tensor_scalar_mul(out, t, 1.0)`
    instructions: the engine's RTL mode-detect sees fp16/step-1/SBUF/
    single-src and picks the 4x program — 4 elements/cycle/lane."""
    import concourse.dve_ops as dve_ops
    from concourse.dve_spec import Spec, Src0, sq
    from concourse.dve_uop import DveOpSpec

    for op in dve_ops.OPS:
        if op.name == CUBE_OP_NAME:
            return op

    spec = Spec(
        body=sq(Src0) * Src0,
        reference=lambda in0, in1, s0, s1, imm2: in0.astype(np.float32) ** 3,
    )
    u1, u2, u2p, u4 = _build_cube_uops()
    compiled = DveOpSpec(
        name=CUBE_OP_NAME,
        opcode=TS_ARITH_OPCODE,
        uops=[u1],
        uops_2x=[u2],
        uops_2x_2p=[u2p],
        uops_4x=[u4],
        perf_max=3,
        rd1_en=False,
    )
    compiled.validate("v3")
    op = dve_ops.DveOp(
        CUBE_OP_NAME, spec, subdim=False, uops_sha={"v3": compiled.sha("v3")}
    )
    dve_ops.OPS.append(op)
    dve_ops.CUSTOM_DVE_SPECS[CUBE_OP_NAME] = spec
    dve_ops._SUB_OPCODE_FOR_NAME[CUBE_OP_NAME] = 0x1F  # unused ant row
    # Pre-seed the compile cache so DveOp.compile() (used by the NEFF DVE
    # table writer) returns the spec with the mode variants + 0x43 row.
    dve_ops._COMPILE_CACHE[(CUBE_OP_NAME, "v3")] = compiled
    return op


def _build():
    import concourse.bacc as bacc
    import concourse.mybir as mybir
    import concourse.tile as tile

    f32 = mybir.dt.float32
    f16 = mybir.dt.float16
    AF = mybir.ActivationFunctionType

    _register_cube_op()

    nc = bacc.Bacc("TRN2", target_bir_lowering=False, debug=False)
    # aTP packs the K=30 feature panels of groups of 4 i-tiles into row
    # strips at partitions 0-29/32-61/64-93/96-125 so 4 consecutive
    # i-tiles' mm1 matmuls run concurrently in different PE row groups
    # (tile_position row tiling) — the K=30 contraction only needs a
    # quarter of the 128-row array.
    # one packed input tensor (aTP | bTP | ypc-rearranged) instead of
    # three: the NEFF preamble pays ~1us of argument-pointer TENSOR_LOAD
    # per dram tensor, which is pure ramp latency
    X = nc.dram_tensor("X", [P, (N // 2) + JL + ITILES * 8], f16, kind="ExternalInput")
    S = nc.dram_tensor("S", [8, JL], f32, kind="ExternalOutput")
    A0, B0, Y0 = 0, N // 2, N // 2 + JL

    NACT = (NH + HPA - 1) // HPA  # 43 ACTIVATE/cube calls (42 full + 1 tail)

    with tile.TileContext(nc) as tc:
        with (
            tc.tile_pool(name="const", bufs=1) as cpool,
            tc.tile_pool(name="tp", bufs=4) as tpool,
            tc.tile_pool(name="icdp", bufs=4) as icdpool,
            tc.tile_pool(name="zp", bufs=1, space="PSUM") as zpool,
            tc.tile_pool(name="ps2", bufs=1, space="PSUM") as ps2pool,
            tc.tile_pool(name="scr", bufs=1, space="PSUM") as scrpool,
            tc.tile_pool(name="outp", bufs=1) as opool,
        ):
            # PE warm-up: the memset is the only dependency, so the burst
            # starts as soon as the vector queue comes up and trips the HAM
            # clock gate to 8/8 (2.4 GHz) while the input DMAs land.
            warm_in = cpool.tile([P, 512], f16)
            nc.vector.memset(warm_in[:], 0.0)
            scr = scrpool.tile([P, 512], f32)
            for w in range(8):
                nc.tensor.matmul(
                    scr[:], warm_in[:, 0:128], warm_in[:], start=True, stop=True
                )

            # DMA order tracks first use: the earliest mm1 halves need
            # aTP's first column chunk and bTP's first j-half only.
            aTP_sb = cpool.tile([P, N // 2], f16)
            bTP_sb = cpool.tile([P, JL], f16)
            nc.sync.dma_start(aTP_sb[:, 0 : N // 8], X[:, A0 : A0 + N // 8])
            nc.sync.dma_start(bTP_sb[:, 0:512], X[:, B0 : B0 + 512])
            nc.sync.dma_start(bTP_sb[:, 512:1024], X[:, B0 + 512 : B0 + 1024])
            # ypc before the late aTP chunks: the first mm2's LDWEIGHTS
            # gates on it at ~15us, and a late ypc once stalled the PE
            # queue >3.4us — tripping the HAM re-throttle death spiral.
            ypc_sb = cpool.tile([P, ITILES * 8], f16)
            nc.sync.dma_start(ypc_sb[:], X[:, Y0 : Y0 + ITILES * 8])
            for q in range(1, 4):
                lo, hi = q * (N // 8), (q + 1) * (N // 8)
                nc.sync.dma_start(aTP_sb[:, lo:hi], X[:, A0 + lo : A0 + hi])

            # 6-slot z rotation: two ping-pong tiles of 3 PSUM banks each.
            # Separate tiles (not one [128,3072] buffer) because the WAR
            # write-after-read hazard vs the ACTIVATE reads is tracked at
            # tile granularity: with one tile every mm1 write would wait
            # for the latest ACT call and serialize the whole pipeline.
            zbufA = zpool.tile([P, HPA * 512], f32)
            zbufB = zpool.tile([P, HPA * 512], f32)
            zbufs = [zbufA, zbufB]
            # both j-half accumulators share one PSUM bank: rows 0-7 hold
            # the first half, rows 32-39 the second (col tile_position 32)
            ps2 = ps2pool.tile([P, 512], f32)
            ps2a = ps2[0:8, :]
            ps2b = ps2[32:40, :]

            def emit_pulse_mm():
                # short full-array pulse: keeps HAM activity up through the
                # bubbly fill phase without inflating the cold-clock period
                nc.tensor.matmul(
                    scr[:, 0:128], warm_in[:, 0:128], warm_in[:, 0:128],
                    start=True, stop=True,
                )

            def emit_dummy_mm():
                # Dependency-free filler matmul.  The HAM clock gate
                # re-throttles the PE to 1.2 GHz whenever its busy fraction
                # drops; the real mm1+mm2 work alone leaves the warm-clock PE
                # ~60% busy, which re-throttles it and doubles the PE's share
                # until it becomes the pipeline bottleneck.  Padding the PE
                # queue keeps the duty cycle high at 2.4 GHz — a large net
                # win (measured on the previous revision of this kernel).
                nc.tensor.matmul(
                    scr[:], warm_in[:, 0:128], warm_in[:], start=True, stop=True
                )

            def emit_mm1_half(h):
                # z for half-tile h (i-tile h//2, j-half h%2): one K=30
                # matmul into its z slot.  Row strip cycles with HALF mod 4
                # (each i-tile's feature panel is replicated in two strips)
                # so any 4 consecutive halves overlap in the PE array —
                # including the two j-halves of one i-tile.
                jh = h % 2
                r = h % 4
                lhs = aTP_sb[32 * r : 32 * r + KF, (h // 4) * P : (h // 4 + 1) * P]
                zb = zbufs[(h % NSLOT) // HPA]
                s = h % HPA
                nc.tensor.matmul(
                    zb[:, s * 512 : (s + 1) * 512],
                    lhs,
                    bTP_sb[32 * r : 32 * r + KF, jh * 512 : (jh + 1) * 512],
                    start=True,
                    stop=True,
                    tile_position=(32 * r, 0),
                )

            # icd half-tile h lives at pieces[h] = (tile, column offset)
            pieces: dict[int, tuple] = {}
            next_mm1 = 0  # next half-tile whose mm1 has not been emitted
            next_mm2 = 0  # next i-tile whose mm2 has not been emitted

            # fill: z for the first ACT call plus the other ping-pong half
            while next_mm1 < NSLOT:
                emit_mm1_half(next_mm1)
                next_mm1 += 1

            for k in range(NACT):
                h0 = k * HPA
                nh = min(HPA, NH - h0)  # 3, except 2 on the tail call
                fd = nh * 512
                t_sb = tpool.tile([P, HPA * 512], f16, tag="t")
                nc.scalar.activation(
                    t_sb[:, 0:fd],
                    zbufs[k % 2][:, 0:fd],
                    AF.Abs_reciprocal_sqrt,
                    scale=ACT_SCALE,
                )
                icd = icdpool.tile([P, HPA * 512], f16, tag="icd")
                # lowers to opcode 0x43, whose table row now holds the cube
                nc.vector.tensor_scalar_mul(icd[:, 0:fd], t_sb[:, 0:fd], 1.0)
                for i in range(nh):
                    pieces[h0 + i] = (icd, i * 512)
                # fill only the ping-pong half freed by ACT call k-1: a
                # deeper lookahead would queue matmuls that WAR-depend on
                # the ACT call just issued, and the in-order PE queue then
                # head-of-line-blocks every instruction behind them.  They
                # go ahead of the dummy/mm2 emissions: the next ACT call
                # gates on them.
                while next_mm1 < NH and next_mm1 <= h0 + nh - 1 + HPA:
                    emit_mm1_half(next_mm1)
                    next_mm1 += 1
                # mm2 for every i-tile whose both halves now have icd
                while next_mm2 < ITILES and 2 * next_mm2 + 1 <= h0 + nh - 1:
                    t = next_mm2
                    ia, oa = pieces.pop(2 * t)
                    ib, ob = pieces.pop(2 * t + 1)
                    first, last = t == 0, t == ITILES - 1
                    nc.tensor.matmul(
                        ps2a,
                        ypc_sb[:, t * 8 : (t + 1) * 8],
                        ia[:, oa : oa + 512],
                        start=first,
                        stop=last,
                        tile_position=(0, 0),
                    )
                    nc.tensor.matmul(
                        ps2b,
                        ypc_sb[:, t * 8 : (t + 1) * 8],
                        ib[:, ob : ob + 512],
                        start=first,
                        stop=last,
                        tile_position=(0, 32),
                    )
                    next_mm2 += 1
                emit_dummy_mm()
                if k < 8:
                    # denser padding through the pipeline-fill phase: the
                    # HAM gate is most likely to re-throttle in the bubbly
                    # transition right after the warm-up burst
                    emit_pulse_mm()
                    emit_pulse_mm()

            S_sb = opool.tile([8, JL], f32)
            nc.vector.tensor_copy(S_sb[:, 0:512], ps2a)
            nc.sync.dma_start(S[:, 0:512], S_sb[:, 0:512])
            nc.vector.tensor_copy(S_sb[:, 512:1024], ps2b)
            nc.sync.dma_start(S[:, 512:1024], S_sb[:, 512:1024])
    # the cube rides the stock tensor_scalar opcode, so no custom-DVE
    # instruction records the op name — attach it explicitly so the NEFF
    # DVE table writer emits our 0x43 row.
    nc.m.ant_custom_dve_ops = sorted({*nc.m.ant_custom_dve_ops, CUBE_OP_NAME})
    nc.compile()
    return nc


def _split16(x):
    hi = x.astype(np.float16)
    lo = (x - hi.astype(np.float32)).astype(np.float16)
    return hi, lo


def _split16_3(x):
    h = x.astype(np.float16)
    r = x - h.astype(np.float32)
    m = r.astype(np.float16)
    l = (r - m.astype(np.float32)).astype(np.float16)
    return h, m, l


def kernel(t, y, masses, G):
    global LAST_RUN
    from concourse.bass_utils import run_bass_kernel_spmd

    y = np.asarray(y, np.float32).reshape(N, 3)
    m = np.asarray(masses, np.float32).reshape(N)
    g = np.float32(np.asarray(G).reshape(()))

    d2 = (y * y).sum(1, dtype=np.float32)
    ones = np.ones(N, np.float32)
    a = np.stack([y[:, 0], y[:, 1], y[:, 2], d2, ones])  # [5, N] fp32
    b = np.stack([-2 * y[:, 0], -2 * y[:, 1], -2 * y[:, 2], ones, d2 + EPS])
    ah, am, al = _split16_3(a)
    bh, bm, bl = _split16_3(b)
    # (ah+am+al).(bh+bm+bl) expanded, keeping pairs whose product can reach
    # ~2^-22 of z: (h,h) (h,m) (m,h) (h,l) (l,h) (m,m); dropped terms < 2^-33.
    aT30 = np.concatenate([ah, ah, am, ah, al, am], axis=0)  # [30, N]
    bT30_full = np.concatenate([bh, bm, bh, bl, bh, bm], axis=0)  # [30, N]
    # pack the feature panel for half-tile h into row strip h%4, column
    # block h//4 (partitions 0-29 / 32-61 / 64-93 / 96-125); each i-tile's
    # panel lands in two strips, one per j-half, so any 4 consecutive
    # halves run concurrently in the PE array
    NHALF = 2 * ITILES
    aTP = np.zeros((P, (NHALF // 4) * P), np.float16)
    aTP_v = aTP.reshape(P, NHALF // 4, P)
    for h in range(NHALF):
        r, blk = h % 4, h // 4
        t = h // 2
        aTP_v[32 * r : 32 * r + KF, blk] = aT30[:, t * P : (t + 1) * P]
    yp = np.concatenate([y, ones[:, None]], axis=1)  # [N, 4] fp32
    yph, ypl = _split16(yp)
    ypc = np.ascontiguousarray(np.concatenate([yph, ypl], axis=1))  # [N, 8]
    # pre-rearranged for the packed input: ypc_r[p, t*8+c] = ypc[t*128+p, c]
    ypc_r = ypc.reshape(ITILES, P, 8).transpose(1, 0, 2).reshape(P, ITILES * 8)

    if "nc" not in _cache:
        _cache["nc"] = _build()
    nc = _cache["nc"]

    in_maps = []
    for c in range(NCORES):
        bT_loc = bT30_full[:, c * JL : (c + 1) * JL]
        bTP = np.zeros((P, JL), np.float16)
        for r in range(4):
            bTP[32 * r : 32 * r + KF] = bT_loc
        X = np.concatenate([aTP, bTP, ypc_r], axis=1)
        in_maps.append({"X": np.ascontiguousarray(X)})
    LAST_RUN = run_bass_kernel_spmd(nc, in_maps, core_ids=list(range(NCORES)))
    S8 = np.concatenate([r["S"] for r in LAST_RUN.results], axis=1)  # [8, N]
    S = S8[0:4] + S8[4:8]
    acc = (np.float32(ICD_SCALE) * g * m)[:, None] * (
        S[0:3].T - y * S[3][:, None]
    )
    return acc.astype(np.float32)
